# revision 38
# baseline (speedup 1.0000x reference)
"""MoE (top-2 of 8 experts, SwiGLU) Trainium2 kernel.

Strategy: expert-parallel across 8 NeuronCores — core e owns expert e's
weights. Routing (gate matmul, softmax, top-k) runs on host in fp64 (tiny:
[4096,8]); tokens are gathered per expert on host, padded to a common
capacity C, and each core computes its expert's SwiGLU MLP over its token
batch in bf16 with fp32 PSUM accumulation. The top-k combine weight is
folded into the PSUM->SBUF eviction of the final matmul, so the host-side
combine is just two gathers and an add.

Layout on device is token-transposed [feature, token] so all three matmuls
keep the natural lhsT layout:
  g.T[I,C]  = Wg[H,I].T-matmul  with lhsT=Wg tiles,  rhs=x.T tiles
  mid[I,C]  = silu(g.T) * u.T
  y.T[H,C]  = Wd[I,H].T-matmul  with lhsT=Wd tiles,  rhs=mid tiles
"""

import numpy as np
import ml_dtypes

B, S, H, I, E = 2, 2048, 1024, 2048, 8
T = B * S
P = 128
KH = H // P     # 8  k-tiles over H (contraction of matmul 1/2)
MI = I // P     # 16 m-tiles over I
KI = I // P     # 16 k-tiles over I (contraction of matmul 3)
MH = H // P     # 8  m-tiles over H
NTILE = 512     # tokens per moving-operand tile (one PSUM bank of fp32)
MCHUNK = 8      # m-tiles computed concurrently (8 PSUM banks)

_kernel_cache = {}


def _round_up(v, m):
    return ((v + m - 1) // m) * m


def _build(C):
    import concourse.mybir as mybir
    import concourse.tile as tile
    from concourse import bacc

    bf16 = mybir.dt.bfloat16
    f32 = mybir.dt.float32
    nc = bacc.Bacc("TRN2", target_bir_lowering=False, debug=False, num_devices=E)

    HI = I // 2  # 1024
    WA = I + C  # packed k-tile row: [Wg_k(m 0..7) | xT_k | Wg_k(m 8..15)]
    pka_d = nc.dram_tensor("pka", [KH, P, WA], bf16, kind="ExternalInput")
    pkb_d = nc.dram_tensor("pkb", [KH, P, I], bf16, kind="ExternalInput")
    wd = nc.dram_tensor("wd", [KI, P, H], bf16, kind="ExternalInput")
    wrow = nc.dram_tensor("wrow", [P, C], f32, kind="ExternalInput")
    yt = nc.dram_tensor("yt", [MH, P, C], f32, kind="ExternalOutput")

    # descending token-tile sizes: lead with 512 (slow phase hides the
    # input-DMA pipeline), keep every tile >=176 so none is PE-issue-floor
    # bound (stream time >= ~56ns/MM floor), end small for a short tail
    sizes = []
    rem = C
    while rem > NTILE:
        if rem - NTILE >= 176:
            nxt = NTILE
        else:
            nxt = (rem - 176) // 16 * 16
        sizes.append(nxt)
        rem -= nxt
    sizes.append(rem)
    n_tiles = []
    off = 0
    for n in sizes:
        n_tiles.append((off, n))
        off += n
    assert off == C and all(0 < n <= NTILE for _, n in n_tiles)

    with tile.TileContext(nc) as tc:
        with (
            tc.tile_pool(name="inp", bufs=1) as inp,
            tc.tile_pool(name="work", bufs=1) as work,
            tc.tile_pool(name="yout", bufs=8) as yout,
            tc.tile_pool(name="ps", bufs=1, space="PSUM") as psp,
        ):
            from concourse.tile_rust import add_dep_helper

            # chain input DMAs so tiles complete in consumption order at
            # full (single-transfer) bandwidth instead of fair-sharing
            # across 8 queues and all completing at the end
            dma_chain = []

            def chained_dma(dst, src):
                # stride-3 chain: three transfers in flight hides the ~2.5us
                # sem->trigger->first-byte latency while keeping arrival
                # roughly in consumption order
                d = nc.sync.dma_start(dst, src)
                if len(dma_chain) >= 4:
                    add_dep_helper(d.ins, dma_chain[-4].ins, True,
                                   "pipeline input DMA arrivals")
                dma_chain.append(d)

            n_first = n_tiles[0][1]
            w1 = HI + n_first  # Wg m0..7 + x columns of the first token tile
            pka = []
            for k in range(KH):
                t = inp.tile([P, WA], bf16, name=f"pka{k}", tag=f"pka{k}")
                if k == 0:
                    # latency-critical first tile: split across both HWDGE
                    # rings (SP + ACT) so descriptor-gen and transfer overlap
                    h = w1 // 2
                    nc.scalar.dma_start(t[:, :h], pka_d[k][:, :h])
                    chained_dma(t[:, h:w1], pka_d[k][:, h:w1])
                else:
                    chained_dma(t[:, :w1], pka_d[k][:, :w1])
                pka.append(t)
            for k in range(KH):
                chained_dma(pka[k][:, w1:], pka_d[k][:, w1:])
            pkb = []
            for k in range(KH):
                t = inp.tile([P, I], bf16, name=f"pkb{k}", tag=f"pkb{k}")
                chained_dma(t[:], pkb_d[k])
                pkb.append(t)
            wdt = inp.tile([P, KI, H], bf16, name="wdt")
            wd_r = wd.rearrange("k p h -> p k h")
            chained_dma(wdt[:, :KI // 2], wd_r[:, :KI // 2])
            chained_dma(wdt[:, KI // 2:], wd_r[:, KI // 2:])
            wr = inp.tile([P, C], f32, name="wr")
            chained_dma(wr[:], wrow[:])

            def wg_slice(k, m):
                if m < MI // 2:
                    return pka[k][:, m * P:(m + 1) * P]
                return pka[k][:, HI + C + (m - MI // 2) * P:
                              HI + C + (m - MI // 2 + 1) * P]

            for n0, n in n_tiles:
                xs = [t[:, HI + n0:HI + n0 + n] for t in pka]

                # stage A1: g = Wg.T @ x, silu -> sg (bf16)
                sgs = []
                for mc in range(MI // MCHUNK):
                    pgs = [
                        psp.tile([P, n], f32, name=f"bank{j}", tag=f"bank{j}")
                        for j in range(MCHUNK)
                    ]
                    for k in range(KH):
                        for j in range(MCHUNK):
                            m = mc * MCHUNK + j
                            nc.tensor.matmul(
                                pgs[j][:],
                                wg_slice(k, m),
                                xs[k],
                                start=(k == 0),
                                stop=(k == KH - 1),
                            )
                    for j in range(MCHUNK):
                        m = mc * MCHUNK + j
                        s = work.tile([P, n], bf16, name=f"sg{m}", tag=f"sg{m}")
                        nc.scalar.activation(
                            s[:], pgs[j][:], mybir.ActivationFunctionType.Silu
                        )
                        sgs.append(s)

                # stage A2: u = Wu.T @ x, mid = sg * u (bf16)
                mids = []
                for mc in range(MI // MCHUNK):
                    pus = [
                        psp.tile([P, n], f32, name=f"bank{j}", tag=f"bank{j}")
                        for j in range(MCHUNK)
                    ]
                    for k in range(KH):
                        for j in range(MCHUNK):
                            m = mc * MCHUNK + j
                            nc.tensor.matmul(
                                pus[j][:],
                                pkb[k][:, m * P:(m + 1) * P],
                                xs[k],
                                start=(k == 0),
                                stop=(k == KH - 1),
                            )
                    for j in range(MCHUNK):
                        m = mc * MCHUNK + j
                        md = work.tile([P, n], bf16, name=f"mid{m}", tag=f"mid{m}")
                        nc.vector.tensor_mul(md[:], sgs[m][:], pus[j][:])
                        mids.append(md)

                # stage B: y = Wd.T @ mid, scale by combine weight, DMA out.
                # m2-outer so each output row finishes early and its
                # scale+DMA overlaps the remaining matmuls
                for j in range(MH):
                    py = psp.tile([P, n], f32, name=f"bank{j}", tag=f"bank{j}")
                    for k2 in range(KI):
                        nc.tensor.matmul(
                            py[:],
                            wdt[:, k2, j * P:(j + 1) * P],
                            mids[k2][:],
                            start=(k2 == 0),
                            stop=(k2 == KI - 1),
                        )
                    yo = yout.tile([P, n], f32, name="yo", tag="yo")
                    nc.vector.tensor_mul(yo[:], py[:], wr[:, n0:n0 + n])
                    nc.sync.dma_start(yt[j, :, n0:n0 + n], yo[:])

    nc.compile()
    return nc


def _get_kernel(C):
    nc = _kernel_cache.get(C)
    if nc is None:
        nc = _build(C)
        _kernel_cache[C] = nc
    return nc


_last_run = None  # stashes BassKernelResults for profiling from test harnesses
_input_cache = {}


def _ensure_ntff_hook_stub():
    """If tracing is requested (BASS_TRACE) but this image lacks
    antenv.axon_hooks, install a stub so run_bass_kernel_spmd degrades to
    an untraced run instead of crashing on the import."""
    import os
    if not os.environ.get("BASS_TRACE"):
        return
    try:
        import antenv.axon_hooks  # noqa: F401
    except ImportError:
        import sys
        import types
        mod = types.ModuleType("antenv.axon_hooks")
        mod._hook = None
        mod.set_axon_ntff_profile_hook = lambda h: setattr(mod, "_hook", h)
        mod.get_axon_ntff_profile_hook = lambda: mod._hook
        try:
            import antenv
            sys.modules["antenv.axon_hooks"] = mod
            antenv.axon_hooks = mod
        except ImportError:
            pass


def kernel(hidden_state, gate_w, Wg, Wu, Wd, top_k):
    from concourse.bass_utils import run_bass_kernel_spmd

    global _last_run
    _ensure_ntff_hook_stub()
    bf = ml_dtypes.bfloat16
    # materialize everything as numpy immediately (inputs may be jax arrays)
    x = np.ascontiguousarray(
        np.asarray(hidden_state, dtype=np.float32).reshape(T, H)
    )
    gw = np.asarray(gate_w, dtype=np.float32)
    Wg = np.asarray(Wg, dtype=np.float32)
    Wu = np.asarray(Wu, dtype=np.float32)
    Wd = np.asarray(Wd, dtype=np.float32)
    topk = int(top_k)

    # host routing (fp64 for a stable top-k ranking; jax's fp32 ranking
    # agrees except at <1e-6 prob ties, which don't occur for random data)
    router_logits = (x @ gw).astype(np.float32)
    l64 = x.astype(np.float64) @ gw.astype(np.float64)
    z = np.exp(l64 - l64.max(axis=-1, keepdims=True))
    p64 = z / z.sum(axis=-1, keepdims=True)
    order = np.argsort(-p64, axis=-1, kind="stable")[:, :topk]      # [T,k]
    pvals = np.take_along_axis(p64, order, axis=1)                   # [T,k]

    flat_e = order.ravel()
    flat_t = np.repeat(np.arange(T), topk)
    flat_w = pvals.ravel()
    sort_idx = np.argsort(flat_e, kind="stable")
    counts = np.bincount(flat_e, minlength=E)
    C = _round_up(max(int(counts.max()), NTILE), 16)
    splits = np.cumsum(counts)[:-1]
    toks_by_e = np.split(flat_t[sort_idx], splits)
    ws_by_e = np.split(flat_w[sort_idx], splits)
    # slot of each (t, k) assignment within its expert's batch
    slot_flat = np.empty(T * topk, dtype=np.int64)
    slot_flat[sort_idx] = np.concatenate(
        [np.arange(c, dtype=np.int64) for c in counts]
    )
    slot = slot_flat.reshape(T, topk)

    nc = _get_kernel(C)

    # input buffers: weight halves are call-invariant, so build them once
    # per capacity and only refresh the token-dependent columns per call
    hi = I // 2
    # weight fingerprint so a repeat call with different weights doesn't
    # reuse stale cached input buffers
    fp = (
        C,
        hash(Wg.ravel()[::65537].tobytes()),
        hash(Wu.ravel()[::65537].tobytes()),
        hash(Wd.ravel()[::65537].tobytes()),
    )
    cache = _input_cache.get(fp)
    if cache is None:
        _input_cache.clear()
        Wg_b = Wg.astype(bf).reshape(E, KH, P, I)
        in_maps = []
        for e in range(E):
            pka_in = np.empty((KH, P, I + C), dtype=bf)
            pka_in[:, :, :hi] = Wg_b[e][:, :, :hi]
            pka_in[:, :, hi + C:] = Wg_b[e][:, :, hi:]
            in_maps.append({
                "pka": pka_in,
                "pkb": np.ascontiguousarray(
                    Wu[e].astype(bf).reshape(KH, P, I)),
                "wd": np.ascontiguousarray(
                    Wd[e].astype(bf).reshape(KI, P, H)),
                "wrow": np.zeros((P, C), dtype=np.float32),
            })
        _input_cache[fp] = in_maps
    else:
        in_maps = cache

    for e in range(E):
        toks = toks_by_e[e]
        c = len(toks)
        xe = np.zeros((C, H), dtype=np.float32)
        xe[:c] = x[toks]
        in_maps[e]["pka"][:, :, hi:hi + C] = (
            np.ascontiguousarray(xe.T).astype(bf).reshape(KH, P, C)
        )
        wrow_in = in_maps[e]["wrow"]
        wrow_in[:, :c] = ws_by_e[e].astype(np.float32)
        wrow_in[:, c:] = 0.0

    res = run_bass_kernel_spmd(nc, in_maps, core_ids=list(range(E)))
    _last_run = res

    # combine: y results are already weight-scaled; each token has topk
    # (expert, slot) contributions
    Y = np.stack([res.results[e]["yt"] for e in range(E)])   # [E, MH, P, C]
    G = Y.reshape(E, H, C).transpose(0, 2, 1)                # [E, C, H]
    final = G[order[:, 0], slot[:, 0]]
    for k in range(1, topk):
        final = final + G[order[:, k], slot[:, k]]
    return final.reshape(B, S, H).astype(np.float32), router_logits


# revision 40
# speedup vs baseline: 1.0041x; 1.0041x over previous
"""MoE (top-2 of 8 experts, SwiGLU) Trainium2 kernel.

Strategy: expert-parallel across 8 NeuronCores — core e owns expert e's
weights. Routing (gate matmul, softmax, top-k) runs on host in fp64 (tiny:
[4096,8]); tokens are gathered per expert on host, padded to a common
capacity C, and each core computes its expert's SwiGLU MLP over its token
batch in bf16 with fp32 PSUM accumulation. The top-k combine weight is
folded into the PSUM->SBUF eviction of the final matmul, so the host-side
combine is just two gathers and an add.

Layout on device is token-transposed [feature, token] so all three matmuls
keep the natural lhsT layout:
  g.T[I,C]  = Wg[H,I].T-matmul  with lhsT=Wg tiles,  rhs=x.T tiles
  mid[I,C]  = silu(g.T) * u.T
  y.T[H,C]  = Wd[I,H].T-matmul  with lhsT=Wd tiles,  rhs=mid tiles
"""

import numpy as np
import ml_dtypes

B, S, H, I, E = 2, 2048, 1024, 2048, 8
T = B * S
P = 128
KH = H // P     # 8  k-tiles over H (contraction of matmul 1/2)
MI = I // P     # 16 m-tiles over I
KI = I // P     # 16 k-tiles over I (contraction of matmul 3)
MH = H // P     # 8  m-tiles over H
NTILE = 512     # tokens per moving-operand tile (one PSUM bank of fp32)
MCHUNK = 8      # m-tiles computed concurrently (8 PSUM banks)

_kernel_cache = {}


def _round_up(v, m):
    return ((v + m - 1) // m) * m


def _build(C):
    import concourse.mybir as mybir
    import concourse.tile as tile
    from concourse import bacc

    bf16 = mybir.dt.bfloat16
    f32 = mybir.dt.float32
    nc = bacc.Bacc("TRN2", target_bir_lowering=False, debug=False, num_devices=E)

    HI = I // 2  # 1024
    WA = I + C  # packed k-tile row: [Wg_k(m 0..7) | xT_k | Wg_k(m 8..15)]
    pka_d = nc.dram_tensor("pka", [KH, P, WA], bf16, kind="ExternalInput")
    pkb_d = nc.dram_tensor("pkb", [KH, P, I], bf16, kind="ExternalInput")
    wd = nc.dram_tensor("wd", [KI, P, H], bf16, kind="ExternalInput")
    wrow = nc.dram_tensor("wrow", [P, C], f32, kind="ExternalInput")
    yt = nc.dram_tensor("yt", [MH, P, C], f32, kind="ExternalOutput")

    # descending token-tile sizes: lead with 512 (slow phase hides the
    # input-DMA pipeline), keep every tile >=176 so none is PE-issue-floor
    # bound (stream time >= ~56ns/MM floor), end small for a short tail
    sizes = []
    rem = C
    while rem > NTILE:
        if rem - NTILE >= 176:
            nxt = NTILE
        else:
            nxt = (rem - 176) // 16 * 16
        sizes.append(nxt)
        rem -= nxt
    sizes.append(rem)
    n_tiles = []
    off = 0
    for n in sizes:
        n_tiles.append((off, n))
        off += n
    assert off == C and all(0 < n <= NTILE for _, n in n_tiles)

    with tile.TileContext(nc) as tc:
        with (
            tc.tile_pool(name="inp", bufs=1) as inp,
            tc.tile_pool(name="work", bufs=1) as work,
            tc.tile_pool(name="yout", bufs=8) as yout,
            tc.tile_pool(name="ps", bufs=1, space="PSUM") as psp,
        ):
            from concourse.tile_rust import add_dep_helper

            # chain input DMAs so tiles complete in consumption order at
            # full (single-transfer) bandwidth instead of fair-sharing
            # across 8 queues and all completing at the end
            dma_chain = []

            def chained_dma(dst, src):
                # stride-3 chain: three transfers in flight hides the ~2.5us
                # sem->trigger->first-byte latency while keeping arrival
                # roughly in consumption order
                d = nc.sync.dma_start(dst, src)
                if len(dma_chain) >= 3:
                    add_dep_helper(d.ins, dma_chain[-3].ins, True,
                                   "pipeline input DMA arrivals")
                dma_chain.append(d)

            n_first = n_tiles[0][1]
            w1 = HI + n_first  # Wg m0..7 + x columns of the first token tile
            pka = []
            for k in range(KH):
                t = inp.tile([P, WA], bf16, name=f"pka{k}", tag=f"pka{k}")
                if k == 0:
                    # latency-critical first tile: split across both HWDGE
                    # rings (SP + ACT) so descriptor-gen and transfer overlap
                    h = w1 // 2
                    nc.scalar.dma_start(t[:, :h], pka_d[k][:, :h])
                    chained_dma(t[:, h:w1], pka_d[k][:, h:w1])
                else:
                    chained_dma(t[:, :w1], pka_d[k][:, :w1])
                pka.append(t)
            for k in range(KH):
                chained_dma(pka[k][:, w1:], pka_d[k][:, w1:])
            pkb = []
            for k in range(KH):
                t = inp.tile([P, I], bf16, name=f"pkb{k}", tag=f"pkb{k}")
                chained_dma(t[:], pkb_d[k])
                pkb.append(t)
            wdt = inp.tile([P, KI, H], bf16, name="wdt")
            chained_dma(wdt[:], wd.rearrange("k p h -> p k h"))
            wr = inp.tile([P, C], f32, name="wr")
            chained_dma(wr[:], wrow[:])

            def wg_slice(k, m):
                if m < MI // 2:
                    return pka[k][:, m * P:(m + 1) * P]
                return pka[k][:, HI + C + (m - MI // 2) * P:
                              HI + C + (m - MI // 2 + 1) * P]

            for n0, n in n_tiles:
                xs = [t[:, HI + n0:HI + n0 + n] for t in pka]

                # stage A1: g = Wg.T @ x, silu -> sg (bf16)
                sgs = []
                for mc in range(MI // MCHUNK):
                    pgs = [
                        psp.tile([P, n], f32, name=f"bank{j}", tag=f"bank{j}")
                        for j in range(MCHUNK)
                    ]
                    for k in range(KH):
                        for j in range(MCHUNK):
                            m = mc * MCHUNK + j
                            nc.tensor.matmul(
                                pgs[j][:],
                                wg_slice(k, m),
                                xs[k],
                                start=(k == 0),
                                stop=(k == KH - 1),
                            )
                    for j in range(MCHUNK):
                        m = mc * MCHUNK + j
                        s = work.tile([P, n], bf16, name=f"sg{m}", tag=f"sg{m}")
                        nc.scalar.activation(
                            s[:], pgs[j][:], mybir.ActivationFunctionType.Silu
                        )
                        sgs.append(s)

                # stage A2: u = Wu.T @ x, mid = sg * u (bf16)
                mids = []
                for mc in range(MI // MCHUNK):
                    pus = [
                        psp.tile([P, n], f32, name=f"bank{j}", tag=f"bank{j}")
                        for j in range(MCHUNK)
                    ]
                    for k in range(KH):
                        for j in range(MCHUNK):
                            m = mc * MCHUNK + j
                            nc.tensor.matmul(
                                pus[j][:],
                                pkb[k][:, m * P:(m + 1) * P],
                                xs[k],
                                start=(k == 0),
                                stop=(k == KH - 1),
                            )
                    for j in range(MCHUNK):
                        m = mc * MCHUNK + j
                        md = work.tile([P, n], bf16, name=f"mid{m}", tag=f"mid{m}")
                        nc.vector.tensor_mul(md[:], sgs[m][:], pus[j][:])
                        mids.append(md)

                # stage B: y = Wd.T @ mid, scale by combine weight, DMA out.
                # m2-outer so each output row finishes early and its
                # scale+DMA overlaps the remaining matmuls
                for j in range(MH):
                    py = psp.tile([P, n], f32, name=f"bank{j}", tag=f"bank{j}")
                    for k2 in range(KI):
                        nc.tensor.matmul(
                            py[:],
                            wdt[:, k2, j * P:(j + 1) * P],
                            mids[k2][:],
                            start=(k2 == 0),
                            stop=(k2 == KI - 1),
                        )
                    yo = yout.tile([P, n], f32, name="yo", tag="yo")
                    nc.vector.tensor_mul(yo[:], py[:], wr[:, n0:n0 + n])
                    nc.sync.dma_start(yt[j, :, n0:n0 + n], yo[:])

    nc.compile()
    return nc


def _get_kernel(C):
    nc = _kernel_cache.get(C)
    if nc is None:
        nc = _build(C)
        _kernel_cache[C] = nc
    return nc


_last_run = None  # stashes BassKernelResults for profiling from test harnesses
_input_cache = {}


def _ensure_ntff_hook_stub():
    """If tracing is requested (BASS_TRACE) but this image lacks
    antenv.axon_hooks, install a stub so run_bass_kernel_spmd degrades to
    an untraced run instead of crashing on the import."""
    import os
    if not os.environ.get("BASS_TRACE"):
        return
    try:
        import antenv.axon_hooks  # noqa: F401
    except ImportError:
        import sys
        import types
        mod = types.ModuleType("antenv.axon_hooks")
        mod._hook = None
        mod.set_axon_ntff_profile_hook = lambda h: setattr(mod, "_hook", h)
        mod.get_axon_ntff_profile_hook = lambda: mod._hook
        try:
            import antenv
            sys.modules["antenv.axon_hooks"] = mod
            antenv.axon_hooks = mod
        except ImportError:
            pass


def kernel(hidden_state, gate_w, Wg, Wu, Wd, top_k):
    from concourse.bass_utils import run_bass_kernel_spmd

    global _last_run
    _ensure_ntff_hook_stub()
    bf = ml_dtypes.bfloat16
    # materialize everything as numpy immediately (inputs may be jax arrays)
    x = np.ascontiguousarray(
        np.asarray(hidden_state, dtype=np.float32).reshape(T, H)
    )
    gw = np.asarray(gate_w, dtype=np.float32)
    Wg = np.asarray(Wg, dtype=np.float32)
    Wu = np.asarray(Wu, dtype=np.float32)
    Wd = np.asarray(Wd, dtype=np.float32)
    topk = int(top_k)

    # host routing (fp64 for a stable top-k ranking; jax's fp32 ranking
    # agrees except at <1e-6 prob ties, which don't occur for random data)
    router_logits = (x @ gw).astype(np.float32)
    l64 = x.astype(np.float64) @ gw.astype(np.float64)
    z = np.exp(l64 - l64.max(axis=-1, keepdims=True))
    p64 = z / z.sum(axis=-1, keepdims=True)
    order = np.argsort(-p64, axis=-1, kind="stable")[:, :topk]      # [T,k]
    pvals = np.take_along_axis(p64, order, axis=1)                   # [T,k]

    flat_e = order.ravel()
    flat_t = np.repeat(np.arange(T), topk)
    flat_w = pvals.ravel()
    sort_idx = np.argsort(flat_e, kind="stable")
    counts = np.bincount(flat_e, minlength=E)
    C = _round_up(max(int(counts.max()), NTILE), 16)
    splits = np.cumsum(counts)[:-1]
    toks_by_e = np.split(flat_t[sort_idx], splits)
    ws_by_e = np.split(flat_w[sort_idx], splits)
    # slot of each (t, k) assignment within its expert's batch
    slot_flat = np.empty(T * topk, dtype=np.int64)
    slot_flat[sort_idx] = np.concatenate(
        [np.arange(c, dtype=np.int64) for c in counts]
    )
    slot = slot_flat.reshape(T, topk)

    nc = _get_kernel(C)

    # input buffers: weight halves are call-invariant, so build them once
    # per capacity and only refresh the token-dependent columns per call
    hi = I // 2
    # weight fingerprint so a repeat call with different weights doesn't
    # reuse stale cached input buffers
    fp = (
        C,
        hash(Wg.ravel()[::65537].tobytes()),
        hash(Wu.ravel()[::65537].tobytes()),
        hash(Wd.ravel()[::65537].tobytes()),
    )
    cache = _input_cache.get(fp)
    if cache is None:
        _input_cache.clear()
        Wg_b = Wg.astype(bf).reshape(E, KH, P, I)
        in_maps = []
        for e in range(E):
            pka_in = np.empty((KH, P, I + C), dtype=bf)
            pka_in[:, :, :hi] = Wg_b[e][:, :, :hi]
            pka_in[:, :, hi + C:] = Wg_b[e][:, :, hi:]
            in_maps.append({
                "pka": pka_in,
                "pkb": np.ascontiguousarray(
                    Wu[e].astype(bf).reshape(KH, P, I)),
                "wd": np.ascontiguousarray(
                    Wd[e].astype(bf).reshape(KI, P, H)),
                "wrow": np.zeros((P, C), dtype=np.float32),
            })
        _input_cache[fp] = in_maps
    else:
        in_maps = cache

    for e in range(E):
        toks = toks_by_e[e]
        c = len(toks)
        xe = np.zeros((C, H), dtype=np.float32)
        xe[:c] = x[toks]
        in_maps[e]["pka"][:, :, hi:hi + C] = (
            np.ascontiguousarray(xe.T).astype(bf).reshape(KH, P, C)
        )
        wrow_in = in_maps[e]["wrow"]
        wrow_in[:, :c] = ws_by_e[e].astype(np.float32)
        wrow_in[:, c:] = 0.0

    res = run_bass_kernel_spmd(nc, in_maps, core_ids=list(range(E)))
    _last_run = res

    # combine: y results are already weight-scaled; each token has topk
    # (expert, slot) contributions
    Y = np.stack([res.results[e]["yt"] for e in range(E)])   # [E, MH, P, C]
    G = Y.reshape(E, H, C).transpose(0, 2, 1)                # [E, C, H]
    final = G[order[:, 0], slot[:, 0]]
    for k in range(1, topk):
        final = final + G[order[:, k], slot[:, k]]
    return final.reshape(B, S, H).astype(np.float32), router_logits
